# revision 13
# baseline (speedup 1.0000x reference)
"""DigitCapsule (dynamic routing) Trainium2 Bass kernel — v2.

Problem: x (128,1152,8) f32, W (1,1152,10,16,8) f32 ->
  u_hat[b,r,o,do] = sum_di W[r,o,do,di] x[b,r,di]
  3 routing iterations (softmax over routes r, squash), output v (128,10,16,1).

Sharding: data-parallel over batch, 16 samples per core, W replicated.

Per-core layout (partition p = 16*j + b, j = r mod 8, b = batch-in-core):
  u[p, cc, do, o] = u_hat[b, 8*cc+j, o, do]   (fp16, 144 x 16 x 10 free)

v2 changes vs v1:
  - xd (block-diag stationary) built on host incl. zeros -> plain DMA, no
    19us DVE memset on the critical path.
  - s0 = sum_r u_hat computed directly from (x, W) via 72 k=128 matmuls
    (xs/ws layouts) accumulating while production runs; v0 is squashed on
    16 partitions and broadcast via one matmul.
  - PSUM->SBUF eviction of u round-robins DVE/Act/Pool.
  - squash is elementwise: v = s*|s|/(1+s^2)  (mag_sq in the reference is
    over a singleton axis).  Uses only Square/Abs from the exp_and_others
    activation table -> single LoadActFuncSet with Exp.
  - softmax denominator via 10 per-o Act Exp calls with accum_out.
  - agreement tree all-fp16 (2x DVE mode); some premul groups on Pool.
"""

import numpy as np

import concourse.bacc as bacc
import concourse.bass as bass
import concourse.tile as tile
from concourse import mybir
from concourse.bass_utils import run_bass_kernel_spmd

B, R, O, DO, DI = 128, 1152, 10, 16, 8
NCORES = 8
BC = B // NCORES          # 16 samples per core
J = 8                     # routes per matmul group
CC = R // J               # 144 matmul groups
OD = O * DO               # 160
G0 = 72                   # k=128 chunks for direct s0 (16 r x 8 di each)
F16 = mybir.dt.float16
F32 = mybir.dt.float32
AF = mybir.ActivationFunctionType
ALU = mybir.AluOpType

PROD_BATCH = 2            # cc per production psum batch (1 bank each)
TREE_BATCH = 24           # cc per premult/tree batch
NG = CC // TREE_BATCH     # 6 groups
POOL_AGREE_GROUPS = (5,)  # agreement premul groups on Pool engine
POOL_S_GROUPS = (5,)      # s-premul groups on Pool engine


def _squash_elem(nc, pool, s_ps, v_out, scale, tag):
    """v_out = squash(s_ps * scale) elementwise: v = k2*s*|s| / (1 + (k*s)^2).

    s_ps: [P, DO, O] f32 (PSUM).  v_out: [P, DO, O] (any dtype).
    Uses Act Square/Abs (exp_and_others set) + DVE.
    """
    P = s_ps.shape[0]
    q = pool.tile([P, DO, O], F32, tag=tag + "q")
    ab = pool.tile([P, DO, O], F32, tag=tag + "a")
    d = pool.tile([P, DO, O], F32, tag=tag + "d")
    p1 = pool.tile([P, DO, O], F32, tag=tag + "p")
    nc.scalar.activation(q[:], s_ps[:], AF.Square, scale=float(scale))
    nc.scalar.activation(ab[:], s_ps[:], AF.Abs, scale=float(scale * scale))
    nc.vector.tensor_scalar_add(d[:], q[:], 1.0)
    nc.vector.reciprocal(d[:], d[:])
    nc.vector.tensor_mul(p1[:], s_ps[:], ab[:])
    nc.vector.tensor_mul(v_out[:], p1[:], d[:])


def build_nc():
    nc = bacc.Bacc("TRN2", debug=False)
    wt_d = nc.dram_tensor("wt", [64, CC, DO, O], F16, kind="ExternalInput")
    xd_d = nc.dram_tensor("xd", [64, CC, 128], F16, kind="ExternalInput")
    xs_d = nc.dram_tensor("xs", [128, G0, BC], F16, kind="ExternalInput")
    ws_d = nc.dram_tensor("ws", [128, G0, DO, O], F16, kind="ExternalInput")
    d16_d = nc.dram_tensor("d16", [128, 128], F16, kind="ExternalInput")
    d32_d = nc.dram_tensor("d32", [128, 128], F32, kind="ExternalInput")
    dout_d = nc.dram_tensor("dout", [128, BC], F16, kind="ExternalInput")
    bc16_d = nc.dram_tensor("bc16", [BC, 128], F16, kind="ExternalInput")
    out_d = nc.dram_tensor("out", [BC, O, DO], F32, kind="ExternalOutput")

    with tile.TileContext(nc) as tc:
        with (
            tc.tile_pool(name="const", bufs=1) as const,
            tc.tile_pool(name="prod", bufs=1) as prod,
            tc.tile_pool(name="main", bufs=1) as main,
            tc.tile_pool(name="sq", bufs=1) as sq,
            tc.tile_pool(name="tp", bufs=2) as tp,
            tc.tile_pool(name="l1p", bufs=1) as l1p,
            tc.tile_pool(name="l2p", bufs=1) as l2p,
            tc.tile_pool(name="l3p", bufs=1) as l3p,
            tc.tile_pool(name="l4p", bufs=1) as l4p,
            tc.tile_pool(name="pp", bufs=3, space=bass.MemorySpace.PSUM) as pp,
            tc.tile_pool(name="pss", bufs=1, space=bass.MemorySpace.PSUM) as pss,
            tc.tile_pool(name="psd", bufs=1, space=bass.MemorySpace.PSUM) as psd,
        ):
            d16 = const.tile([128, 128], F16)
            d32 = const.tile([128, 128], F32)
            dout = const.tile([128, BC], F16)
            bc16 = const.tile([BC, 128], F16)
            nc.sync.dma_start(d16[:], d16_d[:])
            nc.sync.dma_start(d32[:], d32_d[:])
            nc.sync.dma_start(dout[:], dout_d[:])
            nc.sync.dma_start(bc16[:], bc16_d[:])

            wt = prod.tile([64, CC, DO, O], F16)
            xd = prod.tile([64, CC, 128], F16)
            xs = prod.tile([128, G0, BC], F16)
            ws = prod.tile([128, G0, DO, O], F16)
            # chunked DMAs so production can start on early cc; xs/ws go
            # right after the first xd/wt chunk so s0 (hence v0) is early
            NCH = 8
            cch = CC // NCH
            nc.sync.dma_start(xd[:, 0:cch, :], xd_d[:, 0:cch, :])
            nc.sync.dma_start(wt[:, 0:cch, :, :], wt_d[:, 0:cch, :, :])
            nc.sync.dma_start(xs[:], xs_d[:])
            nc.sync.dma_start(ws[:], ws_d[:])
            for ch in range(1, NCH):
                sl = slice(ch * cch, (ch + 1) * cch)
                nc.sync.dma_start(xd[:, sl, :], xd_d[:, sl, :])
                nc.sync.dma_start(wt[:, sl, :, :], wt_d[:, sl, :, :])

            u = main.tile([128, CC, DO, O], F16)

            # ---- produce u_hat; s0 accumulates directly from (x, W) ----
            s0_ps = pss.tile([BC, DO, O], F32, tag="s")
            nb = CC // PROD_BATCH
            for g in range(nb):
                ps = pp.tile([128, PROD_BATCH, 512], F32, tag="pp")
                for i in range(PROD_BATCH):
                    cc = g * PROD_BATCH + i
                    nc.tensor.matmul(
                        ps[:, i, 0:OD], xd[:, cc, :], wt[:, cc, :, :],
                        start=True, stop=True,
                    )
                # interleave s0 accumulation (one per batch, 72 total)
                nc.tensor.matmul(
                    s0_ps[:], xs[:, g, :], ws[:, g, :, :],
                    start=(g == 0), stop=(g == nb - 1),
                )
                sl = slice(g * PROD_BATCH, (g + 1) * PROD_BATCH)
                src = ps[:, :, 0:OD].rearrange(
                    "p c (do o) -> p c do o", do=DO)
                if g % 2 == 0:
                    nc.scalar.copy(u[:, sl, :, :], src)
                else:
                    nc.vector.tensor_copy(u[:, sl, :, :], src)

            # ---- iter 0: v0 = squash(s0 / R), broadcast to 128 partitions ----
            v0_16 = main.tile([BC, DO, O], F16)
            _squash_elem(nc, sq, s0_ps, v0_16, 1.0 / R, tag="s0")
            v_ps = pss.tile([128, DO, O], F32, tag="s")
            nc.tensor.matmul(v_ps[:], bc16[:], v0_16[:], start=True, stop=True)
            v = main.tile([128, DO, O], F16)
            nc.vector.tensor_copy(v[:], v_ps[:])

            b_ij = main.tile([128, CC, O], F32)
            e = main.tile([128, CC, O], F32)
            e_r = main.tile([128, O], F32)
            inv = main.tile([128, O], F32)
            c16 = main.tile([128, CC, O], F16)

            for it in (1, 2):
                final = it == 2
                # ---- agreement: b_ij (+)= sum_do u * v; exp per group on Act ----
                for g in range(NG):
                    sl = slice(g * TREE_BATCH, (g + 1) * TREE_BATCH)
                    t = tp.tile([128, TREE_BATCH, DO, O], F16, tag="t")
                    v_b = v[:].unsqueeze(1).broadcast_to((128, TREE_BATCH, DO, O))
                    if g in POOL_AGREE_GROUPS:
                        nc.gpsimd.tensor_mul(t[:], u[:, sl, :, :], v_b)
                    else:
                        nc.vector.tensor_mul(t[:], u[:, sl, :, :], v_b)
                    l1 = l1p.tile([128, TREE_BATCH, 8, O], F16, tag="l1")
                    nc.vector.tensor_add(l1[:], t[:, :, 0:8, :], t[:, :, 8:16, :])
                    l2 = l2p.tile([128, TREE_BATCH, 4, O], F16, tag="l2")
                    nc.vector.tensor_add(l2[:], l1[:, :, 0:4, :], l1[:, :, 4:8, :])
                    l3 = l3p.tile([128, TREE_BATCH, 2, O], F16, tag="l3")
                    nc.vector.tensor_add(l3[:], l2[:, :, 0:2, :], l2[:, :, 2:4, :])
                    if it == 1:
                        nc.vector.tensor_add(
                            b_ij[:, sl, :], l3[:, :, 0, :], l3[:, :, 1, :])
                    else:
                        a4 = l4p.tile([128, TREE_BATCH, O], F16, tag="l4")
                        nc.vector.tensor_add(a4[:], l3[:, :, 0, :], l3[:, :, 1, :])
                        nc.vector.tensor_add(b_ij[:, sl, :], b_ij[:, sl, :], a4[:])
                    # exp of this group's slice overlaps the next group's tree
                    nc.scalar.activation(e[:, sl, :], b_ij[:, sl, :], AF.Exp)

                # ---- softmax denominator ----
                e_perm = e[:].transpose((0, 2, 1))
                nc.vector.reduce_sum(e_r[:], e_perm, axis=mybir.AxisListType.X)
                den = psd.tile([128, O], F32, tag="den")
                nc.tensor.matmul(den[:], d32[:], e_r[:], start=True, stop=True)
                nc.vector.reciprocal(inv[:], den[:])

                # ---- s = sum_r c * u  (c formed per group) ----
                sp_p = BC if final else 128
                lhs = dout if final else d16
                s_ps2 = pss.tile([sp_p, DO, O], F32, tag="s")
                inv_b = inv[:].unsqueeze(1).broadcast_to((128, TREE_BATCH, O))
                for g in range(NG):
                    sl = slice(g * TREE_BATCH, (g + 1) * TREE_BATCH)
                    eng = nc.gpsimd if g in POOL_S_GROUPS else nc.vector
                    eng.tensor_mul(c16[:, sl, :], e[:, sl, :], inv_b)
                    t = tp.tile([128, TREE_BATCH, DO, O], F16, tag="t")
                    c_b = c16[:, sl, :].unsqueeze(2).broadcast_to(
                        (128, TREE_BATCH, DO, O))
                    eng.tensor_mul(t[:], u[:, sl, :, :], c_b)
                    for i in range(TREE_BATCH):
                        cc = g * TREE_BATCH + i
                        nc.tensor.matmul(
                            s_ps2[:], lhs[:, :sp_p], t[:, i, :, :],
                            start=(cc == 0), stop=(cc == CC - 1),
                        )
                if not final:
                    _squash_elem(nc, sq, s_ps2, v, 1.0, tag="sv")
                else:
                    v2 = main.tile([BC, DO, O], F32)
                    _squash_elem(nc, sq, s_ps2, v2, 1.0, tag="sf")
                    v2p = main.tile([BC, O, DO], F32)
                    nc.vector.tensor_copy(v2p[:], v2[:].transpose((0, 2, 1)))
                    nc.sync.dma_start(out_d[:], v2p[:])

    nc.compile()
    return nc


_CACHE = {}


def _get_nc():
    if "nc" not in _CACHE:
        _CACHE["nc"] = build_nc()
    return _CACHE["nc"]


def _prep_const():
    if "const" not in _CACHE:
        p = np.arange(128)
        d16 = (p[:, None] % 16 == p[None, :] % 16).astype(np.float16)
        d32 = d16.astype(np.float32)
        dout = (p[:, None] % 16 == np.arange(BC)[None, :]).astype(np.float16)
        bc16 = (np.arange(BC)[:, None] == p[None, :] % 16).astype(np.float16)
        _CACHE["const"] = (d16, d32, dout, bc16)
    return _CACHE["const"]


def _prep_w(W):
    W5 = np.ascontiguousarray(W.reshape(R, O, DO, DI))
    # wt[8j+di, cc, do, o] = W[8cc+j, o, do, di]
    wt = np.ascontiguousarray(
        W5.reshape(CC, J, O, DO, DI).transpose(1, 4, 0, 3, 2)
    ).reshape(64, CC, DO, O).astype(np.float16)
    # ws[8rr+di, g, do, o] = W[16g+rr, o, do, di]
    ws = np.ascontiguousarray(
        W5.reshape(G0, 16, O, DO, DI).transpose(1, 4, 0, 3, 2)
    ).reshape(128, G0, DO, O).astype(np.float16)
    return wt, ws


def kernel(x: np.ndarray, W: np.ndarray) -> np.ndarray:
    x = np.asarray(x, dtype=np.float32)
    W = np.asarray(W, dtype=np.float32)
    nc = _get_nc()
    d16, d32, dout, bc16 = _prep_const()
    wt, ws = _prep_w(W)
    in_maps = []
    for q in range(NCORES):
        xq = x[BC * q : BC * (q + 1)]           # [16, 1152, 8]
        # xd[8j+di, cc, 16j'+b] = x[b, 8cc+j, di] * (j == j')
        xf = xq.reshape(BC, CC, J, DI).transpose(2, 3, 1, 0)  # [j, di, cc, b]
        xd = np.zeros((J, DI, CC, J, BC), dtype=np.float16)
        for j in range(J):
            xd[j, :, :, j, :] = xf[j]
        xd = np.ascontiguousarray(xd).reshape(64, CC, 128)
        # xs[8rr+di, g, b] = x[b, 16g+rr, di]
        xs = np.ascontiguousarray(
            xq.reshape(BC, G0, 16, DI).transpose(2, 3, 1, 0)
        ).reshape(128, G0, BC).astype(np.float16)
        in_maps.append({
            "wt": wt, "xd": xd, "xs": xs, "ws": ws,
            "d16": d16, "d32": d32, "dout": dout, "bc16": bc16,
        })
    res = run_bass_kernel_spmd(nc, in_maps, core_ids=list(range(NCORES)))
    out = np.concatenate([res.results[q]["out"] for q in range(NCORES)], axis=0)
    return out.reshape(B, O, DO, 1).astype(np.float32)


# revision 15
# speedup vs baseline: 1.0064x; 1.0064x over previous
"""DigitCapsule (dynamic routing) Trainium2 Bass kernel — v2.

Problem: x (128,1152,8) f32, W (1,1152,10,16,8) f32 ->
  u_hat[b,r,o,do] = sum_di W[r,o,do,di] x[b,r,di]
  3 routing iterations (softmax over routes r, squash), output v (128,10,16,1).

Sharding: data-parallel over batch, 16 samples per core, W replicated.

Per-core layout (partition p = 16*j + b, j = r mod 8, b = batch-in-core):
  u[p, cc, do, o] = u_hat[b, 8*cc+j, o, do]   (fp16, 144 x 16 x 10 free)

v2 changes vs v1:
  - xd (block-diag stationary) built on host incl. zeros -> plain DMA, no
    19us DVE memset on the critical path.
  - s0 = sum_r u_hat computed directly from (x, W) via 72 k=128 matmuls
    (xs/ws layouts) accumulating while production runs; v0 is squashed on
    16 partitions and broadcast via one matmul.
  - PSUM->SBUF eviction of u round-robins DVE/Act/Pool.
  - squash is elementwise: v = s*|s|/(1+s^2)  (mag_sq in the reference is
    over a singleton axis).  Uses only Square/Abs from the exp_and_others
    activation table -> single LoadActFuncSet with Exp.
  - softmax denominator via 10 per-o Act Exp calls with accum_out.
  - agreement tree all-fp16 (2x DVE mode); some premul groups on Pool.
"""

import numpy as np

import concourse.bacc as bacc
import concourse.bass as bass
import concourse.tile as tile
from concourse import mybir
from concourse.bass_utils import run_bass_kernel_spmd

B, R, O, DO, DI = 128, 1152, 10, 16, 8
NCORES = 8
BC = B // NCORES          # 16 samples per core
J = 8                     # routes per matmul group
CC = R // J               # 144 matmul groups
OD = O * DO               # 160
G0 = 72                   # k=128 chunks for direct s0 (16 r x 8 di each)
F16 = mybir.dt.float16
F32 = mybir.dt.float32
AF = mybir.ActivationFunctionType
ALU = mybir.AluOpType

PROD_BATCH = 2            # cc per production psum batch (1 bank each)
TREE_BATCH = 24           # cc per premult/tree batch
NG = CC // TREE_BATCH     # 6 groups
POOL_AGREE_GROUPS = (0,)  # agreement premul groups on Pool engine
POOL_S_GROUPS = (0,)      # s-premul groups on Pool engine


def _squash_elem(nc, pool, s_ps, v_out, scale, tag):
    """v_out = squash(s_ps * scale) elementwise: v = k2*s*|s| / (1 + (k*s)^2).

    s_ps: [P, DO, O] f32 (PSUM).  v_out: [P, DO, O] (any dtype).
    Uses Act Square/Abs (exp_and_others set) + DVE.
    """
    P = s_ps.shape[0]
    q = pool.tile([P, DO, O], F32, tag=tag + "q")
    ab = pool.tile([P, DO, O], F32, tag=tag + "a")
    d = pool.tile([P, DO, O], F32, tag=tag + "d")
    p1 = pool.tile([P, DO, O], F32, tag=tag + "p")
    nc.scalar.activation(q[:], s_ps[:], AF.Square, scale=float(scale))
    nc.scalar.activation(ab[:], s_ps[:], AF.Abs, scale=float(scale * scale))
    nc.vector.tensor_scalar_add(d[:], q[:], 1.0)
    nc.vector.reciprocal(d[:], d[:])
    nc.vector.tensor_mul(p1[:], s_ps[:], ab[:])
    nc.vector.tensor_mul(v_out[:], p1[:], d[:])


def build_nc():
    nc = bacc.Bacc("TRN2", debug=False)
    wt_d = nc.dram_tensor("wt", [64, CC, DO, O], F16, kind="ExternalInput")
    xd_d = nc.dram_tensor("xd", [64, CC, 128], F16, kind="ExternalInput")
    xs_d = nc.dram_tensor("xs", [128, G0, BC], F16, kind="ExternalInput")
    ws_d = nc.dram_tensor("ws", [128, G0, DO, O], F16, kind="ExternalInput")
    d16_d = nc.dram_tensor("d16", [128, 128], F16, kind="ExternalInput")
    d32_d = nc.dram_tensor("d32", [128, 128], F32, kind="ExternalInput")
    dout_d = nc.dram_tensor("dout", [128, BC], F16, kind="ExternalInput")
    bc16_d = nc.dram_tensor("bc16", [BC, 128], F16, kind="ExternalInput")
    out_d = nc.dram_tensor("out", [BC, O, DO], F32, kind="ExternalOutput")

    with tile.TileContext(nc) as tc:
        with (
            tc.tile_pool(name="const", bufs=1) as const,
            tc.tile_pool(name="prod", bufs=1) as prod,
            tc.tile_pool(name="main", bufs=1) as main,
            tc.tile_pool(name="sq", bufs=1) as sq,
            tc.tile_pool(name="tp", bufs=2) as tp,
            tc.tile_pool(name="l1p", bufs=1) as l1p,
            tc.tile_pool(name="l2p", bufs=1) as l2p,
            tc.tile_pool(name="l3p", bufs=1) as l3p,
            tc.tile_pool(name="l4p", bufs=1) as l4p,
            tc.tile_pool(name="pp", bufs=3, space=bass.MemorySpace.PSUM) as pp,
            tc.tile_pool(name="pss", bufs=1, space=bass.MemorySpace.PSUM) as pss,
            tc.tile_pool(name="psd", bufs=1, space=bass.MemorySpace.PSUM) as psd,
        ):
            d16 = const.tile([128, 128], F16)
            d32 = const.tile([128, 128], F32)
            dout = const.tile([128, BC], F16)
            bc16 = const.tile([BC, 128], F16)
            nc.sync.dma_start(d16[:], d16_d[:])
            nc.sync.dma_start(d32[:], d32_d[:])
            nc.sync.dma_start(dout[:], dout_d[:])
            nc.sync.dma_start(bc16[:], bc16_d[:])

            wt = prod.tile([64, CC, DO, O], F16)
            xd = prod.tile([64, CC, 128], F16)
            xs = prod.tile([128, G0, BC], F16)
            ws = prod.tile([128, G0, DO, O], F16)
            # chunked DMAs interleaved so production and s0 stream together
            NCH = 8
            cch = CC // NCH
            gch = G0 // NCH
            for ch in range(NCH):
                sl = slice(ch * cch, (ch + 1) * cch)
                nc.sync.dma_start(xd[:, sl, :], xd_d[:, sl, :])
                nc.sync.dma_start(wt[:, sl, :, :], wt_d[:, sl, :, :])
                sg = slice(ch * gch, (ch + 1) * gch)
                nc.sync.dma_start(xs[:, sg, :], xs_d[:, sg, :])
                nc.sync.dma_start(ws[:, sg, :, :], ws_d[:, sg, :, :])

            u = main.tile([128, CC, DO, O], F16)

            # ---- produce u_hat; s0 accumulates directly from (x, W) ----
            s0_ps = pss.tile([BC, DO, O], F32, tag="s")
            nb = CC // PROD_BATCH
            for g in range(nb):
                ps = pp.tile([128, PROD_BATCH, 512], F32, tag="pp")
                for i in range(PROD_BATCH):
                    cc = g * PROD_BATCH + i
                    nc.tensor.matmul(
                        ps[:, i, 0:OD], xd[:, cc, :], wt[:, cc, :, :],
                        start=True, stop=True,
                    )
                # interleave s0 accumulation (one per batch, 72 total)
                nc.tensor.matmul(
                    s0_ps[:], xs[:, g, :], ws[:, g, :, :],
                    start=(g == 0), stop=(g == nb - 1),
                )
                sl = slice(g * PROD_BATCH, (g + 1) * PROD_BATCH)
                src = ps[:, :, 0:OD].rearrange(
                    "p c (do o) -> p c do o", do=DO)
                if g % 2 == 0:
                    nc.scalar.copy(u[:, sl, :, :], src)
                else:
                    nc.vector.tensor_copy(u[:, sl, :, :], src)

            # ---- iter 0: v0 = squash(s0 / R), broadcast to 128 partitions ----
            v0_16 = main.tile([BC, DO, O], F16)
            _squash_elem(nc, sq, s0_ps, v0_16, 1.0 / R, tag="s0")
            v_ps = pss.tile([128, DO, O], F32, tag="s")
            nc.tensor.matmul(v_ps[:], bc16[:], v0_16[:], start=True, stop=True)
            v = main.tile([128, DO, O], F16)
            nc.vector.tensor_copy(v[:], v_ps[:])

            b_ij = main.tile([128, CC, O], F32)
            e = main.tile([128, CC, O], F32)
            e_r = main.tile([128, O], F32)
            inv = main.tile([128, O], F32)
            c16 = main.tile([128, CC, O], F16)

            for it in (1, 2):
                final = it == 2
                # ---- agreement: b_ij (+)= sum_do u * v; exp per group on Act ----
                for g in range(NG):
                    sl = slice(g * TREE_BATCH, (g + 1) * TREE_BATCH)
                    t = tp.tile([128, TREE_BATCH, DO, O], F16, tag="t")
                    v_b = v[:].unsqueeze(1).broadcast_to((128, TREE_BATCH, DO, O))
                    if g in POOL_AGREE_GROUPS:
                        nc.gpsimd.tensor_mul(t[:], u[:, sl, :, :], v_b)
                    else:
                        nc.vector.tensor_mul(t[:], u[:, sl, :, :], v_b)
                    l1 = l1p.tile([128, TREE_BATCH, 8, O], F16, tag="l1")
                    nc.vector.tensor_add(l1[:], t[:, :, 0:8, :], t[:, :, 8:16, :])
                    l2 = l2p.tile([128, TREE_BATCH, 4, O], F16, tag="l2")
                    nc.vector.tensor_add(l2[:], l1[:, :, 0:4, :], l1[:, :, 4:8, :])
                    l3 = l3p.tile([128, TREE_BATCH, 2, O], F16, tag="l3")
                    nc.vector.tensor_add(l3[:], l2[:, :, 0:2, :], l2[:, :, 2:4, :])
                    if it == 1:
                        nc.vector.tensor_add(
                            b_ij[:, sl, :], l3[:, :, 0, :], l3[:, :, 1, :])
                    else:
                        a4 = l4p.tile([128, TREE_BATCH, O], F16, tag="l4")
                        nc.vector.tensor_add(a4[:], l3[:, :, 0, :], l3[:, :, 1, :])
                        nc.vector.tensor_add(b_ij[:, sl, :], b_ij[:, sl, :], a4[:])
                    # exp of this group's slice overlaps the next group's tree
                    nc.scalar.activation(e[:, sl, :], b_ij[:, sl, :], AF.Exp)

                # ---- softmax denominator ----
                e_perm = e[:].transpose((0, 2, 1))
                nc.vector.reduce_sum(e_r[:], e_perm, axis=mybir.AxisListType.X)
                den = psd.tile([128, O], F32, tag="den")
                nc.tensor.matmul(den[:], d32[:], e_r[:], start=True, stop=True)
                nc.vector.reciprocal(inv[:], den[:])

                # ---- s = sum_r c * u  (c formed per group) ----
                sp_p = BC if final else 128
                lhs = dout if final else d16
                s_ps2 = pss.tile([sp_p, DO, O], F32, tag="s")
                inv_b = inv[:].unsqueeze(1).broadcast_to((128, TREE_BATCH, O))
                for g in range(NG):
                    sl = slice(g * TREE_BATCH, (g + 1) * TREE_BATCH)
                    eng = nc.gpsimd if g in POOL_S_GROUPS else nc.vector
                    eng.tensor_mul(c16[:, sl, :], e[:, sl, :], inv_b)
                    t = tp.tile([128, TREE_BATCH, DO, O], F16, tag="t")
                    c_b = c16[:, sl, :].unsqueeze(2).broadcast_to(
                        (128, TREE_BATCH, DO, O))
                    eng.tensor_mul(t[:], u[:, sl, :, :], c_b)
                    for i in range(TREE_BATCH):
                        cc = g * TREE_BATCH + i
                        nc.tensor.matmul(
                            s_ps2[:], lhs[:, :sp_p], t[:, i, :, :],
                            start=(cc == 0), stop=(cc == CC - 1),
                        )
                if not final:
                    _squash_elem(nc, sq, s_ps2, v, 1.0, tag="sv")
                else:
                    v2 = main.tile([BC, DO, O], F32)
                    _squash_elem(nc, sq, s_ps2, v2, 1.0, tag="sf")
                    v2p = main.tile([BC, O, DO], F32)
                    nc.vector.tensor_copy(v2p[:], v2[:].transpose((0, 2, 1)))
                    nc.sync.dma_start(out_d[:], v2p[:])

    nc.compile()
    return nc


_CACHE = {}


def _get_nc():
    if "nc" not in _CACHE:
        _CACHE["nc"] = build_nc()
    return _CACHE["nc"]


def _prep_const():
    if "const" not in _CACHE:
        p = np.arange(128)
        d16 = (p[:, None] % 16 == p[None, :] % 16).astype(np.float16)
        d32 = d16.astype(np.float32)
        dout = (p[:, None] % 16 == np.arange(BC)[None, :]).astype(np.float16)
        bc16 = (np.arange(BC)[:, None] == p[None, :] % 16).astype(np.float16)
        _CACHE["const"] = (d16, d32, dout, bc16)
    return _CACHE["const"]


def _prep_w(W):
    W5 = np.ascontiguousarray(W.reshape(R, O, DO, DI))
    # wt[8j+di, cc, do, o] = W[8cc+j, o, do, di]
    wt = np.ascontiguousarray(
        W5.reshape(CC, J, O, DO, DI).transpose(1, 4, 0, 3, 2)
    ).reshape(64, CC, DO, O).astype(np.float16)
    # ws[8rr+di, g, do, o] = W[16g+rr, o, do, di]
    ws = np.ascontiguousarray(
        W5.reshape(G0, 16, O, DO, DI).transpose(1, 4, 0, 3, 2)
    ).reshape(128, G0, DO, O).astype(np.float16)
    return wt, ws


def kernel(x: np.ndarray, W: np.ndarray) -> np.ndarray:
    x = np.asarray(x, dtype=np.float32)
    W = np.asarray(W, dtype=np.float32)
    nc = _get_nc()
    d16, d32, dout, bc16 = _prep_const()
    wt, ws = _prep_w(W)
    in_maps = []
    for q in range(NCORES):
        xq = x[BC * q : BC * (q + 1)]           # [16, 1152, 8]
        # xd[8j+di, cc, 16j'+b] = x[b, 8cc+j, di] * (j == j')
        xf = xq.reshape(BC, CC, J, DI).transpose(2, 3, 1, 0)  # [j, di, cc, b]
        xd = np.zeros((J, DI, CC, J, BC), dtype=np.float16)
        for j in range(J):
            xd[j, :, :, j, :] = xf[j]
        xd = np.ascontiguousarray(xd).reshape(64, CC, 128)
        # xs[8rr+di, g, b] = x[b, 16g+rr, di]
        xs = np.ascontiguousarray(
            xq.reshape(BC, G0, 16, DI).transpose(2, 3, 1, 0)
        ).reshape(128, G0, BC).astype(np.float16)
        in_maps.append({
            "wt": wt, "xd": xd, "xs": xs, "ws": ws,
            "d16": d16, "d32": d32, "dout": dout, "bc16": bc16,
        })
    res = run_bass_kernel_spmd(nc, in_maps, core_ids=list(range(NCORES)))
    out = np.concatenate([res.results[q]["out"] for q in range(NCORES)], axis=0)
    return out.reshape(B, O, DO, 1).astype(np.float32)


# revision 20
# speedup vs baseline: 1.0273x; 1.0208x over previous
"""DigitCapsule (dynamic routing) Trainium2 Bass kernel — v4.

Problem: x (128,1152,8) f32, W (1,1152,10,16,8) f32 ->
  u_hat[b,r,o,do] = sum_di W[r,o,do,di] x[b,r,di]
  3 routing iterations (softmax over routes r, squash), output v (128,10,16,1).

Sharding: data-parallel over batch, 16 samples per core, W replicated.

Per-core layout (partition p = 16*j + b, j = r mod 8, b = batch-in-core):
  u[p, cc, do, o] = u_hat[b, 8*cc+j, o, do]   (fp16, 144 x 16 x 10 free)

Key structure:
  - xd (block-diag x stationary) built on host incl. zeros -> plain DMA
    (input DMA 5.4 MB total; the DMA stream paces the production phase).
  - u produced by 144 matmuls; PSUM->SBUF eviction alternates DVE/Act.
  - s0 = sum_r u via the delta-matrix chain (d16) interleaved into the
    production stream with a 2-batch lag (PE is in-order).
  - squash is elementwise: v = s*|s|/(1+s^2)  (mag_sq in the reference is
    over the trailing singleton axis).  Only Exp/Abs/Square activation
    functions are used -> a single LoadActFuncSet.
  - agreement premul+tree all-fp16 (2x DVE mode); Pool runs group 0's
    premul+tree and the per-group softmax partial sums.
  - exp per group on Act, overlapped with the agreement.
  - s-chain accumulation runs groups in order [1..5, 0] so the slow Pool
    group is consumed last; dummy matmuls pre-warm the PE p-state during
    the softmax window.
"""

import numpy as np

import concourse.bacc as bacc
import concourse.bass as bass
import concourse.tile as tile
from concourse import mybir
from concourse.bass_utils import run_bass_kernel_spmd

B, R, O, DO, DI = 128, 1152, 10, 16, 8
NCORES = 8
BC = B // NCORES          # 16 samples per core
J = 8                     # routes per matmul group
CC = R // J               # 144 matmul groups
OD = O * DO               # 160
F16 = mybir.dt.float16
F32 = mybir.dt.float32
AF = mybir.ActivationFunctionType
ALU = mybir.AluOpType

PROD_BATCH = 2            # cc per production psum batch (1 bank each)
TREE_BATCH = 24           # cc per premult/tree batch
NG = CC // TREE_BATCH     # 6 groups
POOL_GROUP = 0            # premul/tree group owned by the Pool engine
N_WARM = 7                # PE warm-up dummy matmuls per routing iteration


def _squash_elem(nc, pool, s_ps, v_out, scale, tag):
    """v_out = squash(s_ps * scale) elementwise: v = k2*s*|s| / (1 + (k*s)^2)."""
    P = s_ps.shape[0]
    q = pool.tile([P, DO, O], F32, tag=tag + "q")
    ab = pool.tile([P, DO, O], F32, tag=tag + "a")
    d = pool.tile([P, DO, O], F32, tag=tag + "d")
    p1 = pool.tile([P, DO, O], F32, tag=tag + "p")
    nc.scalar.activation(q[:], s_ps[:], AF.Square, scale=float(scale))
    nc.scalar.activation(ab[:], s_ps[:], AF.Abs, scale=float(scale * scale))
    nc.vector.tensor_scalar_add(d[:], q[:], 1.0)
    nc.vector.reciprocal(d[:], d[:])
    nc.vector.tensor_mul(p1[:], s_ps[:], ab[:])
    nc.vector.tensor_mul(v_out[:], p1[:], d[:])


def build_nc():
    nc = bacc.Bacc("TRN2", debug=False)
    wt_d = nc.dram_tensor("wt", [64, CC, DO, O], F16, kind="ExternalInput")
    xd_d = nc.dram_tensor("xd", [64, CC, 128], F16, kind="ExternalInput")
    d16_d = nc.dram_tensor("d16", [128, 128], F16, kind="ExternalInput")
    d32_d = nc.dram_tensor("d32", [128, 128], F32, kind="ExternalInput")
    dout_d = nc.dram_tensor("dout", [128, BC], F16, kind="ExternalInput")
    out_d = nc.dram_tensor("out", [BC, O, DO], F32, kind="ExternalOutput")

    with tile.TileContext(nc) as tc:
        with (
            tc.tile_pool(name="const", bufs=1) as const,
            tc.tile_pool(name="prod", bufs=1) as prod,
            tc.tile_pool(name="main", bufs=1) as main,
            tc.tile_pool(name="sq", bufs=1) as sq,
            tc.tile_pool(name="tp", bufs=3) as tp,
            tc.tile_pool(name="l1p", bufs=2) as l1p,
            tc.tile_pool(name="l2p", bufs=2) as l2p,
            tc.tile_pool(name="l3p", bufs=2) as l3p,
            tc.tile_pool(name="l4p", bufs=2) as l4p,
            tc.tile_pool(name="pp", bufs=3, space=bass.MemorySpace.PSUM) as pp,
            tc.tile_pool(name="pss", bufs=1, space=bass.MemorySpace.PSUM) as pss,
            tc.tile_pool(name="psd", bufs=1, space=bass.MemorySpace.PSUM) as psd,
        ):
            d16 = const.tile([128, 128], F16)
            d32 = const.tile([128, 128], F32)
            dout = const.tile([128, BC], F16)
            nc.sync.dma_start(d16[:], d16_d[:])
            nc.sync.dma_start(d32[:], d32_d[:])
            nc.sync.dma_start(dout[:], dout_d[:])

            wt = prod.tile([64, CC, DO, O], F16)
            xd = prod.tile([64, CC, 128], F16)
            NCH = 8
            cch = CC // NCH
            for ch in range(NCH):
                sl = slice(ch * cch, (ch + 1) * cch)
                nc.sync.dma_start(xd[:, sl, :], xd_d[:, sl, :])
                nc.sync.dma_start(wt[:, sl, :, :], wt_d[:, sl, :, :])

            u = main.tile([128, CC, DO, O], F16)

            # ---- produce u_hat; s0 chain follows two batches behind ----
            s0_ps = pss.tile([128, DO, O], F32, tag="s")
            nb = CC // PROD_BATCH
            for g in range(nb + 2):
                if g < nb:
                    ps = pp.tile([128, PROD_BATCH, 512], F32, tag="pp")
                    for i in range(PROD_BATCH):
                        cc = g * PROD_BATCH + i
                        nc.tensor.matmul(
                            ps[:, i, 0:OD], xd[:, cc, :], wt[:, cc, :, :],
                            start=True, stop=True,
                        )
                if g >= 2:
                    for i in range(PROD_BATCH):
                        cc = (g - 2) * PROD_BATCH + i
                        nc.tensor.matmul(
                            s0_ps[:], d16[:], u[:, cc, :, :],
                            start=(cc == 0), stop=(cc == CC - 1),
                        )
                if g < nb:
                    sl = slice(g * PROD_BATCH, (g + 1) * PROD_BATCH)
                    src = ps[:, :, 0:OD].rearrange(
                        "p c (do o) -> p c do o", do=DO)
                    if g % 2 == 0:
                        nc.scalar.copy(u[:, sl, :, :], src)
                    else:
                        nc.vector.tensor_copy(u[:, sl, :, :], src)

            # ---- iter 0: v0 = squash(s0 / R) (already broadcast) ----
            v = main.tile([128, DO, O], F16)
            _squash_elem(nc, sq, s0_ps, v, 1.0 / R, tag="sv")

            b_ij = main.tile([128, CC, O], F32)
            e = main.tile([128, CC, O], F32)
            e_r = main.tile([128, O], F32)
            inv = main.tile([128, O], F32)
            c16 = main.tile([128, CC, O], F16)

            for it in (1, 2):
                final = it == 2
                # ---- agreement: b_ij (+)= sum_do u * v ----
                l3_last = None
                for g in range(NG):
                    sl = slice(g * TREE_BATCH, (g + 1) * TREE_BATCH)
                    eng = nc.gpsimd if g == POOL_GROUP else nc.vector
                    t = tp.tile([128, TREE_BATCH, DO, O], F16, tag="t")
                    v_b = v[:].unsqueeze(1).broadcast_to((128, TREE_BATCH, DO, O))
                    eng.tensor_mul(t[:], u[:, sl, :, :], v_b)
                    l1 = l1p.tile([128, TREE_BATCH, 8, O], F16, tag="l1")
                    eng.tensor_add(l1[:], t[:, :, 0:8, :], t[:, :, 8:16, :])
                    l2 = l2p.tile([128, TREE_BATCH, 4, O], F16, tag="l2")
                    eng.tensor_add(l2[:], l1[:, :, 0:4, :], l1[:, :, 4:8, :])
                    l3 = l3p.tile([128, TREE_BATCH, 2, O], F16, tag="l3")
                    eng.tensor_add(l3[:], l2[:, :, 0:2, :], l2[:, :, 2:4, :])
                    if it == 1:
                        eng.tensor_add(
                            b_ij[:, sl, :], l3[:, :, 0, :], l3[:, :, 1, :])
                    else:
                        a4 = l4p.tile([128, TREE_BATCH, O], F16, tag="l4")
                        eng.tensor_add(a4[:], l3[:, :, 0, :], l3[:, :, 1, :])
                        eng.tensor_add(b_ij[:, sl, :], b_ij[:, sl, :], a4[:])
                    if g != POOL_GROUP:
                        l3_last = l3
                    # exp of this group overlaps the next group's tree (Act)
                    nc.scalar.activation(e[:, sl, :], b_ij[:, sl, :], AF.Exp)

                # ---- softmax denominator ----
                e_perm = e[:].transpose((0, 2, 1))
                nc.vector.reduce_sum(e_r[:], e_perm, axis=mybir.AxisListType.X)
                # PE p-state warm-up during the softmax window
                warm = psd.tile([128, 512], F32, tag="den")
                for w in range(N_WARM):
                    nc.tensor.matmul(
                        warm[:, 0:480],
                        d16[:], l3_last[:].rearrange("p c x o -> p (c x o)"),
                        start=True, stop=True)
                den = psd.tile([128, 512], F32, tag="den")
                nc.tensor.matmul(den[:, 0:O], d32[:], e_r[:], start=True, stop=True)
                nc.vector.reciprocal(inv[:], den[:, 0:O])

                # ---- s = sum_r c * u  (Pool group last in the psum chain) ----
                sp_p = BC if final else 128
                lhs = dout if final else d16
                s_ps2 = pss.tile([sp_p, DO, O], F32, tag="s")
                inv_b = inv[:].unsqueeze(1).broadcast_to((128, TREE_BATCH, O))
                order = [g for g in range(NG) if g != POOL_GROUP] + [POOL_GROUP]
                for k, g in enumerate(order):
                    sl = slice(g * TREE_BATCH, (g + 1) * TREE_BATCH)
                    eng = nc.gpsimd if g == POOL_GROUP else nc.vector
                    eng.tensor_mul(c16[:, sl, :], e[:, sl, :], inv_b)
                    t = tp.tile([128, TREE_BATCH, DO, O], F16, tag="t")
                    c_b = c16[:, sl, :].unsqueeze(2).broadcast_to(
                        (128, TREE_BATCH, DO, O))
                    eng.tensor_mul(t[:], u[:, sl, :, :], c_b)
                    for i in range(TREE_BATCH):
                        nc.tensor.matmul(
                            s_ps2[:], lhs[:, :sp_p], t[:, i, :, :],
                            start=(k == 0 and i == 0),
                            stop=(k == NG - 1 and i == TREE_BATCH - 1),
                        )
                if not final:
                    _squash_elem(nc, sq, s_ps2, v, 1.0, tag="sv")
                else:
                    v2 = main.tile([BC, DO, O], F32)
                    _squash_elem(nc, sq, s_ps2, v2, 1.0, tag="sf")
                    v2p = main.tile([BC, O, DO], F32)
                    nc.vector.tensor_copy(v2p[:], v2[:].transpose((0, 2, 1)))
                    nc.sync.dma_start(out_d[:], v2p[:])

    nc.compile()
    return nc


_CACHE = {}


def _get_nc():
    if "nc" not in _CACHE:
        _CACHE["nc"] = build_nc()
    return _CACHE["nc"]


def _prep_const():
    if "const" not in _CACHE:
        p = np.arange(128)
        d16 = (p[:, None] % 16 == p[None, :] % 16).astype(np.float16)
        d32 = d16.astype(np.float32)
        dout = (p[:, None] % 16 == np.arange(BC)[None, :]).astype(np.float16)
        _CACHE["const"] = (d16, d32, dout)
    return _CACHE["const"]


def _prep_w(W):
    W5 = np.ascontiguousarray(W.reshape(R, O, DO, DI))
    # wt[8j+di, cc, do, o] = W[8cc+j, o, do, di]
    wt = np.ascontiguousarray(
        W5.reshape(CC, J, O, DO, DI).transpose(1, 4, 0, 3, 2)
    ).reshape(64, CC, DO, O).astype(np.float16)
    return wt


def kernel(x: np.ndarray, W: np.ndarray) -> np.ndarray:
    x = np.asarray(x, dtype=np.float32)
    W = np.asarray(W, dtype=np.float32)
    nc = _get_nc()
    d16, d32, dout = _prep_const()
    wt = _prep_w(W)
    in_maps = []
    for q in range(NCORES):
        xq = x[BC * q : BC * (q + 1)]           # [16, 1152, 8]
        # xd[8j+di, cc, 16j'+b] = x[b, 8cc+j, di] * (j == j')
        xf = xq.reshape(BC, CC, J, DI).transpose(2, 3, 1, 0)  # [j, di, cc, b]
        xd = np.zeros((J, DI, CC, J, BC), dtype=np.float16)
        for j in range(J):
            xd[j, :, :, j, :] = xf[j]
        xd = np.ascontiguousarray(xd).reshape(64, CC, 128)
        in_maps.append({
            "wt": wt, "xd": xd, "d16": d16, "d32": d32, "dout": dout,
        })
    res = run_bass_kernel_spmd(nc, in_maps, core_ids=list(range(NCORES)))
    out = np.concatenate([res.results[q]["out"] for q in range(NCORES)], axis=0)
    return out.reshape(B, O, DO, 1).astype(np.float32)


# revision 26
# speedup vs baseline: 1.2195x; 1.1871x over previous
"""DigitCapsule (dynamic routing) Trainium2 Bass kernel — v4.

Problem: x (128,1152,8) f32, W (1,1152,10,16,8) f32 ->
  u_hat[b,r,o,do] = sum_di W[r,o,do,di] x[b,r,di]
  3 routing iterations (softmax over routes r, squash), output v (128,10,16,1).

Sharding: data-parallel over batch, 16 samples per core, W replicated.

Per-core layout (partition p = 16*j + b, j = r mod 8, b = batch-in-core):
  u[p, cc, do, o] = u_hat[b, 8*cc+j, o, do]   (fp16, 144 x 16 x 10 free)

Key structure:
  - xd (block-diag x stationary) built on host incl. zeros -> plain DMA
    (input DMA 5.4 MB total; the DMA stream paces the production phase).
  - u produced by 144 matmuls; PSUM->SBUF eviction alternates DVE/Act.
  - s0 = sum_r u via the delta-matrix chain (d16) interleaved into the
    production stream with a 2-batch lag (PE is in-order).
  - squash is elementwise: v = s*|s|/(1+s^2)  (mag_sq in the reference is
    over the trailing singleton axis).  Only Exp/Abs/Square activation
    functions are used -> a single LoadActFuncSet.
  - agreement premul+tree all-fp16 (2x DVE mode); Pool runs group 0's
    premul+tree and the per-group softmax partial sums.
  - exp per group on Act, overlapped with the agreement.
  - s-chain accumulation runs groups in order [1..5, 0] so the slow Pool
    group is consumed last; dummy matmuls pre-warm the PE p-state during
    the softmax window.
"""

import numpy as np

import concourse.bacc as bacc
import concourse.bass as bass
import concourse.tile as tile
from concourse import mybir
from concourse.bass_utils import run_bass_kernel_spmd

B, R, O, DO, DI = 128, 1152, 10, 16, 8
NCORES = 8
BC = B // NCORES          # 16 samples per core
J = 8                     # routes per matmul group
CC = R // J               # 144 matmul groups
OD = O * DO               # 160
F16 = mybir.dt.float16
F32 = mybir.dt.float32
AF = mybir.ActivationFunctionType
ALU = mybir.AluOpType

PROD_BATCH = 2            # cc per production psum batch (1 bank each)
TREE_BATCH = 24           # cc per premult/tree batch
NG = CC // TREE_BATCH     # 6 groups
POOL_GROUP = 0            # premul/tree group owned by the Pool engine
N_WARM = 7                # PE warm-up dummy matmuls per routing iteration


def _tl(pool, shape, tag):
    tile_h = pool.tile(shape, F16, tag=tag, name=tag)
    return tile_h


def _squash_elem(nc, pool, s_ps, v_out, scale, tag):
    """v_out = squash(s_ps * scale) elementwise: v = k2*s*|s| / (1 + (k*s)^2)."""
    P = s_ps.shape[0]
    q = pool.tile([P, DO, O], F32, tag=tag + "q")
    ab = pool.tile([P, DO, O], F32, tag=tag + "a")
    d = pool.tile([P, DO, O], F32, tag=tag + "d")
    p1 = pool.tile([P, DO, O], F32, tag=tag + "p")
    nc.scalar.activation(q[:], s_ps[:], AF.Square, scale=float(scale))
    nc.scalar.activation(ab[:], s_ps[:], AF.Abs, scale=float(scale * scale))
    nc.vector.tensor_scalar_add(d[:], q[:], 1.0)
    nc.vector.reciprocal(d[:], d[:])
    nc.vector.tensor_mul(p1[:], s_ps[:], ab[:])
    nc.vector.tensor_mul(v_out[:], p1[:], d[:])


def build_nc():
    nc = bacc.Bacc("TRN2", debug=False)
    wt_d = nc.dram_tensor("wt", [64, CC, DO, O], F16, kind="ExternalInput")
    xd_d = nc.dram_tensor("xd", [64, CC, 128], F16, kind="ExternalInput")
    d16_d = nc.dram_tensor("d16", [128, 128], F16, kind="ExternalInput")
    d32_d = nc.dram_tensor("d32", [128, 128], F32, kind="ExternalInput")
    dout_d = nc.dram_tensor("dout", [128, BC], F16, kind="ExternalInput")
    out_d = nc.dram_tensor("out", [BC, O, DO], F32, kind="ExternalOutput")

    with tile.TileContext(nc) as tc:
        with (
            tc.tile_pool(name="const", bufs=1) as const,
            tc.tile_pool(name="prod", bufs=1) as prod,
            tc.tile_pool(name="main", bufs=1) as main,
            tc.tile_pool(name="sq", bufs=1) as sq,
            tc.tile_pool(name="tp", bufs=3) as tp,
            tc.tile_pool(name="l1p", bufs=2) as l1p,
            tc.tile_pool(name="l2p", bufs=2) as l2p,
            tc.tile_pool(name="l3p", bufs=2) as l3p,
            tc.tile_pool(name="l4p", bufs=2) as l4p,
            tc.tile_pool(name="pb", bufs=1) as pb,
            tc.tile_pool(name="pp", bufs=3, space=bass.MemorySpace.PSUM) as pp,
            tc.tile_pool(name="pss", bufs=1, space=bass.MemorySpace.PSUM) as pss,
            tc.tile_pool(name="psd", bufs=1, space=bass.MemorySpace.PSUM) as psd,
        ):
            d16 = const.tile([128, 128], F16)
            d32 = const.tile([128, 128], F32)
            dout = const.tile([128, BC], F16)
            nc.sync.dma_start(d16[:], d16_d[:])
            nc.sync.dma_start(d32[:], d32_d[:])
            nc.sync.dma_start(dout[:], dout_d[:])

            NCH = 8
            cch = CC // NCH
            xd_t, wt_t = [], []
            for ch in range(NCH):
                sl = slice(ch * cch, (ch + 1) * cch)
                xd_c = prod.tile([64, cch, 128], F16, tag=f"xd{ch}")
                wt_c = prod.tile([64, cch, DO, O], F16, tag=f"wt{ch}")
                nc.sync.dma_start(xd_c[:], xd_d[:, sl, :])
                nc.sync.dma_start(wt_c[:], wt_d[:, sl, :, :])
                xd_t.append(xd_c)
                wt_t.append(wt_c)

            u = main.tile([128, CC, DO, O], F16)

            # ---- produce u_hat; s0 chain follows two batches behind ----
            s0_ps = pss.tile([128, DO, O], F32, tag="s")
            nb = CC // PROD_BATCH
            for g in range(nb + 2):
                if g < nb:
                    ps = pp.tile([128, PROD_BATCH, 512], F32, tag="pp")
                    for i in range(PROD_BATCH):
                        cc = g * PROD_BATCH + i
                        ch, ci = cc // cch, cc % cch
                        nc.tensor.matmul(
                            ps[:, i, 0:OD], xd_t[ch][:, ci, :],
                            wt_t[ch][:, ci, :, :],
                            start=True, stop=True,
                        )
                if g >= 2:
                    for i in range(PROD_BATCH):
                        cc = (g - 2) * PROD_BATCH + i
                        nc.tensor.matmul(
                            s0_ps[:], d16[:], u[:, cc, :, :],
                            start=(cc == 0), stop=(cc == CC - 1),
                        )
                if g < nb:
                    sl = slice(g * PROD_BATCH, (g + 1) * PROD_BATCH)
                    src = ps[:, :, 0:OD].rearrange(
                        "p c (do o) -> p c do o", do=DO)
                    if g % 2 == 0:
                        nc.scalar.copy(u[:, sl, :, :], src)
                    else:
                        nc.vector.tensor_copy(u[:, sl, :, :], src)

            # ---- iter 0: v0 = squash(s0 / R) (already broadcast) ----
            v = main.tile([128, DO, O], F16)
            _squash_elem(nc, sq, s0_ps, v, 1.0 / R, tag="sv")

            b_ij = main.tile([128, CC, O], F32)
            e = main.tile([128, CC, O], F32)
            e_r = main.tile([128, O], F32)
            inv = main.tile([128, O], F32)
            c16 = main.tile([128, CC, O], F16)

            for it in (1, 2):
                final = it == 2
                # ---- agreement: b_ij (+)= sum_do u * v ----
                l3_last = None
                for g in range(NG):
                    sl = slice(g * TREE_BATCH, (g + 1) * TREE_BATCH)
                    pool_g = g == POOL_GROUP
                    eng = nc.gpsimd if pool_g else nc.vector
                    sfx = "P" if pool_g else ""
                    tpool = pb if pool_g else tp
                    t = tpool.tile([128, TREE_BATCH, DO, O], F16, tag="t" + sfx)
                    v_b = v[:].unsqueeze(1).broadcast_to((128, TREE_BATCH, DO, O))
                    eng.tensor_mul(t[:], u[:, sl, :, :], v_b)
                    l1 = _tl(pb if pool_g else l1p, [128, TREE_BATCH, 8, O], "l1" + sfx)
                    eng.tensor_add(l1[:], t[:, :, 0:8, :], t[:, :, 8:16, :])
                    l2 = _tl(pb if pool_g else l2p, [128, TREE_BATCH, 4, O], "l2" + sfx)
                    eng.tensor_add(l2[:], l1[:, :, 0:4, :], l1[:, :, 4:8, :])
                    l3 = _tl(pb if pool_g else l3p, [128, TREE_BATCH, 2, O], "l3" + sfx)
                    eng.tensor_add(l3[:], l2[:, :, 0:2, :], l2[:, :, 2:4, :])
                    if it == 1:
                        eng.tensor_add(
                            b_ij[:, sl, :], l3[:, :, 0, :], l3[:, :, 1, :])
                    else:
                        a4 = _tl(pb if pool_g else l4p, [128, TREE_BATCH, O], "l4" + sfx)
                        eng.tensor_add(a4[:], l3[:, :, 0, :], l3[:, :, 1, :])
                        eng.tensor_add(b_ij[:, sl, :], b_ij[:, sl, :], a4[:])
                    if not pool_g:
                        l3_last = l3
                    # exp of this group overlaps the next group's tree (Act)
                    nc.scalar.activation(e[:, sl, :], b_ij[:, sl, :], AF.Exp)

                # ---- softmax denominator ----
                e_perm = e[:].transpose((0, 2, 1))
                nc.vector.reduce_sum(e_r[:], e_perm, axis=mybir.AxisListType.X)
                # PE p-state warm-up during the softmax window
                warm = psd.tile([128, 512], F32, tag="den")
                for w in range(N_WARM):
                    nc.tensor.matmul(
                        warm[:, 0:480],
                        d16[:], l3_last[:].rearrange("p c x o -> p (c x o)"),
                        start=True, stop=True)
                den = psd.tile([128, 512], F32, tag="den")
                nc.tensor.matmul(den[:, 0:O], d32[:], e_r[:], start=True, stop=True)
                nc.vector.reciprocal(inv[:], den[:, 0:O])

                # ---- s = sum_r c * u  (Pool group last in the psum chain) ----
                sp_p = BC if final else 128
                lhs = dout if final else d16
                s_ps2 = pss.tile([sp_p, DO, O], F32, tag="s")
                inv_b = inv[:].unsqueeze(1).broadcast_to((128, TREE_BATCH, O))
                order = [POOL_GROUP] + [g for g in range(NG) if g != POOL_GROUP]
                t_tiles = {}
                for k, g in enumerate(order):
                    sl = slice(g * TREE_BATCH, (g + 1) * TREE_BATCH)
                    pool_g = g == POOL_GROUP
                    eng = nc.gpsimd if pool_g else nc.vector
                    eng.tensor_mul(c16[:, sl, :], e[:, sl, :], inv_b)
                    if pool_g:
                        t = pb.tile([128, TREE_BATCH, DO, O], F16, tag="tP")
                    else:
                        t = tp.tile([128, TREE_BATCH, DO, O], F16, tag="t")
                    c_b = c16[:, sl, :].unsqueeze(2).broadcast_to(
                        (128, TREE_BATCH, DO, O))
                    eng.tensor_mul(t[:], u[:, sl, :, :], c_b)
                    t_tiles[g] = t
                    if pool_g:
                        continue
                    first = k == 1
                    for i in range(TREE_BATCH):
                        nc.tensor.matmul(
                            s_ps2[:], lhs[:, :sp_p], t[:, i, :, :],
                            start=(first and i == 0), stop=False,
                        )
                tpg = t_tiles[POOL_GROUP]
                for i in range(TREE_BATCH):
                    nc.tensor.matmul(
                        s_ps2[:], lhs[:, :sp_p], tpg[:, i, :, :],
                        start=False, stop=(i == TREE_BATCH - 1),
                    )
                if not final:
                    _squash_elem(nc, sq, s_ps2, v, 1.0, tag="sv")
                else:
                    v2 = main.tile([BC, DO, O], F32)
                    _squash_elem(nc, sq, s_ps2, v2, 1.0, tag="sf")
                    v2p = main.tile([BC, O, DO], F32)
                    nc.vector.tensor_copy(v2p[:], v2[:].transpose((0, 2, 1)))
                    nc.sync.dma_start(out_d[:], v2p[:])

    nc.compile()
    return nc


_CACHE = {}


def _get_nc():
    if "nc" not in _CACHE:
        _CACHE["nc"] = build_nc()
    return _CACHE["nc"]


def _prep_const():
    if "const" not in _CACHE:
        p = np.arange(128)
        d16 = (p[:, None] % 16 == p[None, :] % 16).astype(np.float16)
        d32 = d16.astype(np.float32)
        dout = (p[:, None] % 16 == np.arange(BC)[None, :]).astype(np.float16)
        _CACHE["const"] = (d16, d32, dout)
    return _CACHE["const"]


def _prep_w(W):
    W5 = np.ascontiguousarray(W.reshape(R, O, DO, DI))
    # wt[8j+di, cc, do, o] = W[8cc+j, o, do, di]
    wt = np.ascontiguousarray(
        W5.reshape(CC, J, O, DO, DI).transpose(1, 4, 0, 3, 2)
    ).reshape(64, CC, DO, O).astype(np.float16)
    return wt


def kernel(x: np.ndarray, W: np.ndarray) -> np.ndarray:
    x = np.asarray(x, dtype=np.float32)
    W = np.asarray(W, dtype=np.float32)
    nc = _get_nc()
    d16, d32, dout = _prep_const()
    wt = _prep_w(W)
    in_maps = []
    for q in range(NCORES):
        xq = x[BC * q : BC * (q + 1)]           # [16, 1152, 8]
        # xd[8j+di, cc, 16j'+b] = x[b, 8cc+j, di] * (j == j')
        xf = xq.reshape(BC, CC, J, DI).transpose(2, 3, 1, 0)  # [j, di, cc, b]
        xd = np.zeros((J, DI, CC, J, BC), dtype=np.float16)
        for j in range(J):
            xd[j, :, :, j, :] = xf[j]
        xd = np.ascontiguousarray(xd).reshape(64, CC, 128)
        in_maps.append({
            "wt": wt, "xd": xd, "d16": d16, "d32": d32, "dout": dout,
        })
    res = run_bass_kernel_spmd(nc, in_maps, core_ids=list(range(NCORES)))
    out = np.concatenate([res.results[q]["out"] for q in range(NCORES)], axis=0)
    return out.reshape(B, O, DO, 1).astype(np.float32)
